# revision 7
# baseline (speedup 1.0000x reference)
"""Trainium2 Bass kernel for nn_ExpandEvecs.

Computes, for evecs [B=4, C=1, M=1024, K=32] and max_lvl=16, the stack of
cumulative low-rank reconstructions
    out[b, l] = V[:, :l+1] @ V[:, :l+1]^T      (V = evecs[b, 0, :, :max_lvl])
returned as [B, max_lvl, M, M] float32 (256 MiB) — a purely output-DMA-bound
problem (~32 MiB written per core across 8 cores).

Sharding: core i handles batch b = i//2 and row-half h = i%2 (512 rows of
every level's M x M matrix).

Precision trick: on the host each eigenvector value v is split as
v = H + E with H = fp16(v), E = fp16(v - H) (22 mantissa bits total). The
level-l Gram matrix is
    sum_{k<=l} v_k v_k^T ~= sum_{k<=l} (H_k H_k^T + H_k E_k^T + E_k H_k^T)
(the dropped E E^T term is ~2^-22 relative). With rows interleaved as
lhsT rows [H_k, E_k, H_k] and rhs rows [H_k, H_k, E_k], ONE fp16 matmul of
contraction 3(l+1) <= 48 computes all three terms, streaming at the full
1 col/cycle PE rate, accumulating exactly in fp32 PSUM.

Per core the kernel is then just: 2 tiny input DMAs; per level l, 8 matmuls
lhsT=vt3_rows[0:3l+3, 128-block], rhs=vt3_full[0:3l+3, 512-chunk] -> PSUM;
PSUM->SBUF copies alternating VectorE/ScalarE; 1 MiB output DMAs alternating
between the two HWDGE rings (sync / scalar).
"""

import sys

for _p in ("/root/.axon_site/_ro/trn_rl_repo", "/opt/trn_rl_repo"):
    if _p not in sys.path:
        sys.path.insert(0, _p)

import numpy as np

import concourse.bacc as bacc
import concourse.mybir as mybir
from concourse.tile import TileContext
from concourse import bass_utils

B, C, M, K, L = 4, 1, 1024, 32, 16
HALF = M // 2
P = 128
R3 = 3 * L  # 48 interleaved rows
F32 = mybir.dt.float32
F16 = mybir.dt.float16

OUT_BUFS = 8


def build_nc(out_bufs=OUT_BUFS):
    nc = bacc.Bacc("TRN2", target_bir_lowering=False, debug=False)
    vt3_full = nc.dram_tensor("vt3_full", [R3, M], F16, kind="ExternalInput")
    vt3_rows = nc.dram_tensor("vt3_rows", [R3, HALF], F16, kind="ExternalInput")
    out = nc.dram_tensor("out", [L, HALF, M], F32, kind="ExternalOutput")

    # out viewed as [L, P, mb, n]: row (mb*128 + p) of a level lives on
    # partition p at free offset mb*1024.
    out_r = out.ap().rearrange("l (mb p) n -> l p mb n", p=P)

    with TileContext(nc) as tc:
        with (
            tc.tile_pool(name="consts", bufs=1) as consts,
            tc.tile_pool(name="outp", bufs=out_bufs) as outp,
            tc.tile_pool(name="psum", bufs=2, space="PSUM") as psump,
        ):
            vt_r = consts.tile([R3, HALF], F16)
            nc.scalar.dma_start(out=vt_r, in_=vt3_rows.ap())
            vt_f = consts.tile([R3, M], F16)
            nc.sync.dma_start(out=vt_f, in_=vt3_full.ap())

            # First levels run fine-grained (256 KiB chunks straight to DMA)
            # so output bandwidth ramps as early as possible; later levels use
            # 1 MiB half-level granularity.
            FINE = 2
            for l in range(L):
                r = 3 * (l + 1)
                for h in range(2):
                    pt = psump.tile([P, 2048], F32)
                    for j in range(4):
                        mb = h * 2 + j // 2
                        nch = j % 2
                        nc.tensor.matmul(
                            pt[:, j * 512 : (j + 1) * 512],
                            vt_r[0:r, mb * P : (mb + 1) * P],
                            vt_f[0:r, nch * 512 : (nch + 1) * 512],
                            start=True,
                            stop=True,
                        )
                        if l < FINE:
                            otc = outp.tile([P, 512], F32, tag="ot_fine")
                            if j % 2 == 0:
                                nc.vector.tensor_copy(out=otc, in_=pt[:, j * 512 : (j + 1) * 512])
                            else:
                                nc.scalar.copy(out=otc, in_=pt[:, j * 512 : (j + 1) * 512])
                            dma_eng = nc.sync if j % 2 == 0 else nc.scalar
                            dma_eng.dma_start(
                                out=out_r[l][:, mb : mb + 1, nch * 512 : (nch + 1) * 512],
                                in_=otc[:, :].rearrange("p (mb n) -> p mb n", mb=1),
                            )
                    if l >= FINE:
                        ot = outp.tile([P, 2048], F32)
                        if h == 0:
                            nc.vector.tensor_copy(out=ot, in_=pt)
                        else:
                            nc.scalar.copy(out=ot, in_=pt)
                        dma_eng = nc.sync if h == 0 else nc.scalar
                        dma_eng.dma_start(
                            out=out_r[l][:, h * 2 : (h + 1) * 2, :],
                            in_=ot[:, :].rearrange("p (mb n) -> p mb n", n=M),
                        )
    nc.compile()
    return nc


_NC_CACHE = {}


def _get_nc():
    key = OUT_BUFS
    if key not in _NC_CACHE:
        _NC_CACHE[key] = build_nc()
    return _NC_CACHE[key]


def _interleave3(a, b, c):
    """rows [a0,b0,c0,a1,b1,c1,...] from [L, N] each -> [3L, N]."""
    out = np.empty((3 * a.shape[0], a.shape[1]), dtype=a.dtype)
    out[0::3] = a
    out[1::3] = b
    out[2::3] = c
    return out


def make_in_maps(evecs):
    evecs = np.asarray(evecs, dtype=np.float32)
    in_maps = []
    for core in range(8):
        b, h = core // 2, core % 2
        vt = np.ascontiguousarray(evecs[b, 0, :, :L].T)  # [L, M] fp32
        hi = vt.astype(np.float16)
        lo = (vt - hi.astype(np.float32)).astype(np.float16)
        full = _interleave3(hi, hi, lo)  # rhs rows: [H, H, E]
        hr = hi[:, h * HALF : (h + 1) * HALF]
        lr = lo[:, h * HALF : (h + 1) * HALF]
        rows = _interleave3(hr, lr, hr)  # lhsT rows: [H, E, H]
        in_maps.append(
            {
                "vt3_full": np.ascontiguousarray(full),
                "vt3_rows": np.ascontiguousarray(rows),
            }
        )
    return in_maps


def assemble(results):
    full = np.empty((B, L * C, M, M), dtype=np.float32)
    for core in range(8):
        b, h = core // 2, core % 2
        full[b, :, h * HALF : (h + 1) * HALF, :] = results[core]["out"]
    return full


def kernel(evecs, max_lvl):
    assert int(max_lvl) == L, f"kernel hardcodes max_lvl={L}, got {max_lvl}"
    nc = _get_nc()
    res = bass_utils.run_bass_kernel_spmd(nc, make_in_maps(evecs), list(range(8)))
    return assemble(res.results)
